# revision 1
# baseline (speedup 1.0000x reference)
"""MoE layer (top-2 routing, 8 experts) on 8 Trainium2 NeuronCores.

Distribution (per the expert-parallel sharding hint):
  1. Gate launch — data-parallel over tokens: 8 cores x 512 tokens each.
     Every core computes logits = x_c @ Wg (PE), the top-2 mask, and the
     softmax-over-top2 weights (sigmoid of the logit difference) fully on
     device, writing its dense [512, 8] gate_weights shard.
  2. Host dispatch — pure indexing: tokens are gathered per top-k expert id
     (the all-to-all of a real MoE), padded to a fixed capacity C.
  3. FFN launch — expert-parallel: core e holds expert e's W1/b1/W2/b2 and
     computes ye = (gelu(xe @ W1 + b1) @ W2 + b2) * gate_w for its tokens.
  4. Host combine — scatter-add of the per-expert outputs into [T, O]
     (the return all-to-all + weighted combine).

All matmuls run as float32r (fp32 operands in the PE's fast mode,
~2e-4 absmax-relative output error vs the fp32 reference).
"""
import sys
import types
import contextlib

import numpy as np

sys.path.insert(0, "/opt/trn_rl_repo")

# ---- environment shims -----------------------------------------------------
if "antenv.axon_hooks" not in sys.modules:
    _m = types.ModuleType("antenv.axon_hooks")
    _m.get_axon_ntff_profile_hook = lambda: None
    sys.modules["antenv.axon_hooks"] = _m

import bass_rust
import concourse.bass as bass
import concourse.tile as tile
from concourse import mybir
from concourse.alu_op_type import AluOpType
from concourse.bass_utils import run_bass_kernel_spmd

dt = mybir.dt
AF = mybir.ActivationFunctionType


def _legalize_waits(nc, max_waits=1, _ctr=[0]):
    """This walrus build rejects instructions with >1 semaphore wait.
    Move excess waits onto preceding same-engine NoOp carriers (engine
    sequencers are in-order, so this is semantically identical)."""
    for f in nc.m.functions:
        for b in f.blocks:
            insts = b.instructions
            out = []
            dirty = False
            for inst in insts:
                si = inst.sync_info
                if si is not None and si.on_wait is not None \
                        and len(si.on_wait) > max_waits:
                    waits = list(si.on_wait)
                    extra, keep = waits[:-max_waits], waits[-max_waits:]
                    for i in range(0, len(extra), max_waits):
                        _ctr[0] += 1
                        nop = bass_rust.InstNoOp(
                            name=f"I-waitsplit-{_ctr[0]}", ins=[], outs=[])
                        nop.engine = inst.engine
                        nop.sync_info = bass_rust.SyncInfo(
                            on_wait=extra[i:i + max_waits], on_update=[])
                        out.append(nop)
                    si.on_wait = keep
                    dirty = True
                out.append(inst)
            if dirty:
                b.instructions = out
    return nc


# ---- problem constants -----------------------------------------------------
B, S, D = 2, 2048, 768
E, K = 8, 2
H, O = 3072, 768
T = B * S
N_CORES = 8
TC = T // N_CORES
ND = D // 128
NH = H // 128


def _f32r(ap):
    return ap.bitcast(dt.float32r)


def _blocks_for(C):
    """Token blocks: multiples of 128, each in [256, 512], descending, so
    block 0's compute covers the weight-streaming window."""
    blocks = []
    rem = C
    while rem > 640:
        blocks.append(512)
        rem -= 512
    if rem > 512:                       # 640 left
        blocks.extend([384, 256])
    elif rem >= 256:
        blocks.append(rem)
    elif rem == 128:                    # convert trailing 512+128 -> 384+256
        blocks[-1] = 384
        blocks.append(256)
    assert sum(blocks) == C and all(
        256 <= b <= 512 and b % 128 == 0 for b in blocks), blocks
    return blocks


# ---- kernel builders -------------------------------------------------------

def build_gate():
    nc = bass.Bass("TRN2", target_bir_lowering=False, debug=False,
                   num_devices=N_CORES)
    xT_d = nc.dram_tensor("xT", [D, TC], dt.float32, kind="ExternalInput").ap()
    wg_d = nc.dram_tensor("Wg", [D, E], dt.float32, kind="ExternalInput").ap()
    gw_d = nc.dram_tensor("gw", [TC, E], dt.float32, kind="ExternalOutput").ap()
    NT = TC // 128

    with tile.TileContext(nc) as tc:
        with contextlib.ExitStack() as ctx:
            sb = ctx.enter_context(tc.tile_pool(name="sb", bufs=2))
            ps = ctx.enter_context(tc.tile_pool(name="ps", bufs=2, space="PSUM"))
            gp = ctx.enter_context(tc.tile_pool(name="gp", bufs=2))

            xT = sb.tile([128, ND, TC], dt.float32, tag="xT")
            nc.sync.dma_start(out=xT,
                              in_=xT_d.rearrange("(c p) t -> p c t", p=128))
            wg = sb.tile([128, ND, E], dt.float32, tag="wg")
            nc.sync.dma_start(out=wg,
                              in_=wg_d.rearrange("(c p) e -> p c e", p=128))
            gwall = gp.tile([128, NT, E], dt.float32, tag="gwall")

            # all 4 token-tiles' logits accumulate into ONE psum bank
            # (4 groups at disjoint column ranges; only the first group's
            # start clears the bank, later groups overwrite-on-clear-bit)
            psl = ps.tile([128, NT, E], dt.float32)
            for tt in range(NT):
                for d in range(ND):
                    nc.tensor.matmul(psl[:, tt, :],
                                     xT[:, d, tt * 128:(tt + 1) * 128],
                                     wg[:, d, :],
                                     start=(d == 0 and tt == 0),
                                     stop=(d == ND - 1))

            def bcast(v):
                return v.rearrange("p (n o) -> p n o", o=1).broadcast_to(
                    [128, NT, E])

            m1 = gp.tile([128, NT], dt.float32, tag="m1")
            nc.vector.tensor_reduce(out=m1, in_=psl, axis=mybir.AxisListType.X,
                                    op=AluOpType.max)
            mask1 = gp.tile([128, NT, E], dt.float32, tag="mask1")
            nc.vector.tensor_tensor(out=mask1, in0=psl, in1=bcast(m1),
                                    op=AluOpType.is_equal)
            l2 = gp.tile([128, NT, E], dt.float32, tag="l2")
            nc.vector.scalar_tensor_tensor(out=l2, in0=mask1, scalar=-1e30,
                                           in1=psl, op0=AluOpType.mult,
                                           op1=AluOpType.add)
            m2 = gp.tile([128, NT], dt.float32, tag="m2")
            nc.vector.tensor_reduce(out=m2, in_=l2, axis=mybir.AxisListType.X,
                                    op=AluOpType.max)
            diff = gp.tile([128, NT], dt.float32, tag="diff")
            nc.vector.tensor_sub(diff, m1, m2)
            w1 = gp.tile([128, NT], dt.float32, tag="w1")
            nc.scalar.activation(out=w1, in_=diff, func=AF.Sigmoid)
            w2 = gp.tile([128, NT], dt.float32, tag="w2")
            nc.vector.tensor_scalar(out=w2, in0=w1, scalar1=-1.0, scalar2=1.0,
                                    op0=AluOpType.mult, op1=AluOpType.add)
            mask2 = gp.tile([128, NT, E], dt.float32, tag="mask2")
            nc.vector.tensor_tensor(out=mask2, in0=l2, in1=bcast(m2),
                                    op=AluOpType.is_equal)
            nc.vector.tensor_tensor(out=mask1, in0=mask1, in1=bcast(w1),
                                    op=AluOpType.mult)
            nc.vector.tensor_tensor(out=mask2, in0=mask2, in1=bcast(w2),
                                    op=AluOpType.mult)
            nc.vector.tensor_tensor(out=gwall, in0=mask1, in1=mask2,
                                    op=AluOpType.add)
            nc.sync.dma_start(
                out=gw_d.rearrange("(tt p) e -> p tt e", p=128), in_=gwall)
    return nc


def build_ffn(C):
    blocks = _blocks_for(C)
    NB = len(blocks)
    nc = bass.Bass("TRN2", target_bir_lowering=False, debug=False,
                   num_devices=N_CORES)
    xeT_d = nc.dram_tensor("xeT", [D, C], dt.float32, kind="ExternalInput").ap()
    w1_d = nc.dram_tensor("w1", [D, H], dt.float32, kind="ExternalInput").ap()
    w2_d = nc.dram_tensor("w2", [H, O], dt.float32, kind="ExternalInput").ap()
    b1_d = nc.dram_tensor("b1", [H], dt.float32, kind="ExternalInput").ap()
    b2_d = nc.dram_tensor("b2", [O], dt.float32, kind="ExternalInput").ap()
    ge_d = nc.dram_tensor("ge", [C], dt.float32, kind="ExternalInput").ap()
    ye_d = nc.dram_tensor("ye", [C, O], dt.float32, kind="ExternalOutput").ap()

    with tile.TileContext(nc) as tc:
        with contextlib.ExitStack() as ctx:
            w1p = ctx.enter_context(tc.tile_pool(name="w1p", bufs=4 * ND))
            w2p = ctx.enter_context(tc.tile_pool(name="w2p", bufs=8))
            cst = ctx.enter_context(tc.tile_pool(name="cst", bufs=1))
            xep = ctx.enter_context(tc.tile_pool(name="xep", bufs=2))
            htp = ctx.enter_context(tc.tile_pool(name="htp", bufs=4))
            yp = ctx.enter_context(tc.tile_pool(name="yp", bufs=6))
            ps1 = ctx.enter_context(tc.tile_pool(name="ps1", bufs=2, space="PSUM"))
            ps2 = ctx.enter_context(tc.tile_pool(name="ps2", bufs=6, space="PSUM"))

            # DMA emission order ~ issue priority: xe block 0 and b1 first,
            # then interleaved w1 q-batches (in mm1 consumption order) with
            # w2 pieces (in mm2 consumption order).
            TB0 = blocks[0]
            xe0 = xep.tile([128, ND, 512], dt.float32r, tag="xe")
            for d_ in range(ND):
                nc.sync.dma_start(
                    out=xe0[:, d_, 0:TB0],
                    in_=_f32r(xeT_d[d_ * 128:(d_ + 1) * 128, 0:TB0]))
            b1_sb = cst.tile([128, NH], dt.float32, tag="b1")
            nc.sync.dma_start(out=b1_sb,
                              in_=b1_d[:].rearrange("(hc p) -> p hc", p=128))
            w1 = {}
            w2 = []
            for q_ in range(4):
                for d_ in range(ND):
                    t_ = w1p.tile([128, H // 4], dt.float32r, tag="w1",
                                  name=f"w1_{d_}_{q_}")
                    nc.sync.dma_start(
                        out=t_,
                        in_=_f32r(w1_d[d_ * 128:(d_ + 1) * 128,
                                       q_ * (H // 4):(q_ + 1) * (H // 4)]))
                    w1[(d_, q_)] = t_
                for p_ in (2 * q_, 2 * q_ + 1):
                    t_ = w2p.tile([128, 3, O], dt.float32r, tag="w2",
                                  name=f"w2_{p_}")
                    nc.sync.dma_start(
                        out=t_,
                        in_=_f32r(w2_d.rearrange("(c p) o -> p c o", p=128)
                                  [:, p_ * 3:(p_ + 1) * 3, :]))
                    w2.append(t_)
            ge_sb = cst.tile([128, C // 128], dt.float32, tag="ge")
            nc.sync.dma_start(out=ge_sb,
                              in_=ge_d[:].rearrange("(i p) -> p i", p=128))
            b2_sb = cst.tile([128, O], dt.float32, tag="b2")
            nc.sync.dma_start(out=b2_sb, in_=b2_d[:].partition_broadcast(128))

            xe = xe0
            t_off = 0
            for tb in range(NB):
                TB = blocks[tb]
                NTT = TB // 128
                if tb > 0:
                    xe = xep.tile([128, ND, 512], dt.float32r, tag="xe")
                    for d_ in range(ND):
                        nc.sync.dma_start(
                            out=xe[:, d_, 0:TB],
                            in_=_f32r(xeT_d[d_ * 128:(d_ + 1) * 128,
                                            t_off:t_off + TB]))

                py_a = [ps2.tile([128, 512], dt.float32, name=f"pya{tb}_{i}",
                                 tag="pya", bufs=4) for i in range(NTT)]
                py_b = [ps2.tile([128, 512], dt.float32, name=f"pyb{tb}_{i}",
                                 tag="pyb", bufs=2)
                        for i in range((NTT + 1) // 2)]

                # software-pipelined: mm1(h)+gelu(h), then mm2(h-1), so the
                # in-order PE never waits on the gelu it just issued. O is
                # split 512+256; the 256-wide accumulators share PSUM banks
                # pairwise (has_written is per element; only the first
                # group's start clears the bank, the second group's first
                # write lands on cleared bits and overwrites).
                def mm2_step(h, hts_h, NTT=NTT, py_a=py_a, py_b=py_b):
                    for tt in range(NTT):
                        nc.tensor.matmul(
                            py_a[tt], hts_h[:, tt * 128:(tt + 1) * 128],
                            w2[h // 3][:, h % 3, 0:512],
                            start=(h == 0), stop=(h == NH - 1))
                        nc.tensor.matmul(
                            py_b[tt // 2][:, (tt % 2) * 256:(tt % 2) * 256 + 256],
                            hts_h[:, tt * 128:(tt + 1) * 128],
                            w2[h // 3][:, h % 3, 512:768],
                            start=(h == 0 and tt % 2 == 0),
                            stop=(h == NH - 1))

                prev_ht = None
                for h in range(NH):
                    ph = ps1.tile([128, TB], dt.float32, name=f"ph{tb}_{h}",
                                  tag="ph")
                    for d_ in range(ND):
                        nc.tensor.matmul(
                            ph,
                            w1[(d_, h // 6)][:, (h % 6) * 128:(h % 6 + 1) * 128],
                            xe[:, d_, 0:TB],
                            start=(d_ == 0), stop=(d_ == ND - 1))
                    ht = htp.tile([128, TB], dt.float32r, name=f"ht{tb}_{h}",
                                  tag="ht")
                    nc.scalar.activation(out=ht, in_=ph, func=AF.Gelu,
                                         bias=b1_sb[:, h:h + 1], scale=1.0)
                    if prev_ht is not None:
                        mm2_step(h - 1, prev_ht)
                    prev_ht = ht
                mm2_step(NH - 1, prev_ht)

                # epilogue: y = (h@W2)*g + b2*g, one store per token-tile
                for tt in range(NTT):
                    i0 = (t_off + tt * 128) // 128
                    gcol = ge_sb[:, i0:i0 + 1]
                    yfin = yp.tile([128, O], dt.float32, tag="yfin")
                    nc.scalar.activation(out=yfin[:, 0:512], in_=py_a[tt],
                                         func=AF.Copy, bias=0.0, scale=gcol)
                    nc.scalar.activation(
                        out=yfin[:, 512:768],
                        in_=py_b[tt // 2][:, (tt % 2) * 256:(tt % 2) * 256 + 256],
                        func=AF.Copy, bias=0.0, scale=gcol)
                    nc.vector.scalar_tensor_tensor(
                        out=yfin, in0=b2_sb, scalar=gcol, in1=yfin,
                        op0=AluOpType.mult, op1=AluOpType.add)
                    r0 = t_off + tt * 128
                    nc.sync.dma_start(out=ye_d[r0:r0 + 128, :], in_=yfin)
                t_off += TB
    return nc


# ---- host orchestration ----------------------------------------------------

_CACHE = {}


def kernel(x, Wg, W1, b1, W2, b2):
    x = np.asarray(x, np.float32)
    Wg = np.asarray(Wg, np.float32)
    W1 = np.asarray(W1, np.float32)
    b1 = np.asarray(b1, np.float32)
    W2 = np.asarray(W2, np.float32)
    b2 = np.asarray(b2, np.float32)
    x_flat = np.ascontiguousarray(x.reshape(-1, D))
    cores = list(range(N_CORES))

    # ---- launch A: gate (data-parallel over tokens)
    if "gate" not in _CACHE:
        _CACHE["gate"] = _legalize_waits(build_gate())
    gate_nc = _CACHE["gate"]
    gmaps = [{"xT": np.ascontiguousarray(x_flat[c * TC:(c + 1) * TC].T),
              "Wg": Wg} for c in cores]
    gres = run_bass_kernel_spmd(gate_nc, gmaps, cores)
    gw_full = np.concatenate([gres.results[c]["gw"] for c in cores], axis=0)

    # ---- host dispatch: token -> expert indices (pure indexing)
    idx = [np.nonzero(gw_full[:, e])[0] for e in range(E)]
    max_n = max(len(i) for i in idx)
    C = max(1152, -(-max_n // 128) * 128)
    if C % 384 and C < 1152:
        C = 1152

    # ---- launch B: expert-parallel FFN
    key = ("ffn", C)
    if key not in _CACHE:
        _CACHE[key] = _legalize_waits(build_ffn(C))
    ffn_nc = _CACHE[key]
    fmaps = []
    for e in range(E):
        ie = idx[e]
        n = len(ie)
        xeT = np.zeros([D, C], np.float32)
        xeT[:, :n] = x_flat[ie].T
        ge = np.zeros([C], np.float32)
        ge[:n] = gw_full[ie, e]
        fmaps.append({"xeT": xeT,
                      "w1": np.ascontiguousarray(W1[e]),
                      "w2": np.ascontiguousarray(W2[e]),
                      "b1": np.ascontiguousarray(b1[e]),
                      "b2": np.ascontiguousarray(b2[e]),
                      "ge": ge})
    fres = run_bass_kernel_spmd(ffn_nc, fmaps, cores)

    # ---- host combine: weighted scatter-add (return all-to-all)
    out = np.zeros([T, O], np.float32)
    for e in range(E):
        ie = idx[e]
        out[ie] += fres.results[e]["ye"][:len(ie)]

    return out.reshape(B, S, O), gw_full
